# revision 1
# baseline (speedup 1.0000x reference)
"""MiniSTU (spectral transform unit) Trainium2 kernel.

Math: out[b,l,o] = sum_k conv_causal(phi_k, x @ Mp[k])[l,o]
               + sum_k (-1)^(l-t)-weighted conv_causal(phi_k, x @ Mm[k])[l,o]

The FFT convolution of the reference is a causal linear convolution
(n=2048 >= 2L-1), computed here as block-Toeplitz matmuls on the tensor
engine.  The alternating-sign modulation of the minus branch depends only
on (l - t), so it folds entirely into the precomputed Toeplitz weights.

Sharding: the K=24 filters are split 3-per-core across 8 cores (every core
runs the identical program on its own filter slice); the host sums the 8
partial outputs.

Per-core schedule (two passes over output-column halves `oh`):
  for lb in 0..7:                       # output time-block, 128 rows
    Y[lb]  = xT-block @ Mcat            # projection, PSUM over i-chunks
    out[lb] = sum_{tb<=lb,k',sign} W[lb-tb,k',sign] @ Y[tb]   # PSUM accum
All matmuls use float32r (full-speed fp32 path, N>=256).
"""

import os
# Ask the runtime to reset cores on acquisition: recovers from a prior
# process leaving a core in NRT_EXEC_UNIT_UNRECOVERABLE state.
os.environ.setdefault("NEURON_RT_RESET_CORES", "1")

import numpy as np
import concourse.bacc as bacc
import concourse.mybir as mybir
from concourse.tile import TileContext
from concourse.bass_utils import run_bass_kernel_spmd

B, L, I, O, K = 4, 1024, 256, 256, 24
S = 128           # block size
NB = L // S       # 8 time blocks
KPC = 3           # filters per core
N_CORES = 8
F32 = mybir.dt.float32
F32R = mybir.dt.float32r

_cache = {}


def _build_program(reps=1):
    """reps>1 repeats the whole compute (timing experiments only)."""
    nc = bacc.Bacc()
    # [ic, i, b*NB*S]  (xT tiles: col = b*1024 + tb*128 + t)
    xt_d = nc.declare_dram_parameter("xt", [2, S, B * NB * S], F32R, isOutput=False)
    # [ic, i, 1536]    (col = oh*768 + sign*384 + kp*128 + o)
    m_d = nc.declare_dram_parameter("mcat", [2, S, 1536], F32R, isOutput=False)
    # [d, t, 768]      (col = kp*256 + sign*128 + l)
    w_d = nc.declare_dram_parameter("w", [NB, S, 768], F32R, isOutput=False)
    # [oh, lb, l, b*128]  (col = b*128 + o)
    out_d = nc.declare_dram_parameter("out", [2, NB, S, B * S], F32, isOutput=True)

    with TileContext(nc) as tc:
        with tc.tile_pool(name="persist", bufs=1) as persist, \
             tc.tile_pool(name="ypool", bufs=NB + 1) as ypool, \
             tc.tile_pool(name="ostage", bufs=3) as ostage, \
             tc.tile_pool(name="pyp", bufs=3, space="PSUM") as pyp, \
             tc.tile_pool(name="poutp", bufs=2, space="PSUM") as poutp:

            # Per-(ic,oh) M tiles, per-(ic,b) xT tiles, per-d W tiles: fine
            # DMA granularity so the first projection/conv only waits on the
            # chunks it reads.  Issue order = first-use order.
            m_sb = {}
            for oh in range(2):
                for ic in range(2):
                    for ch, w_ in ((0, 512), (1, 256)):
                        m_sb[ic, oh, ch] = persist.tile(
                            [S, w_], F32R, tag=f"m{ic}{oh}{ch}",
                            name=f"m_sb{ic}{oh}{ch}")
            xt_sb = {}
            for b in range(B):
                for ic in range(2):
                    for q in range(NB // 2):
                        xt_sb[ic, b, q] = persist.tile(
                            [S, 2 * S], F32R, tag=f"xt{ic}{b}{q}",
                            name=f"xt_sb{ic}{b}{q}")
            w_sb = {}
            for d in range(NB):
                w_sb[d] = persist.tile(
                    [S, 768], F32R, tag=f"w{d}", name=f"w_sb{d}")

            # First-use-critical loads on HWDGE in exact first-need order;
            # bulk streams on SWDGE (gpsimd).  xt is tiled per (ic, b,
            # lb-pair) so projection lb only waits on its own slices.
            def xt_dma(eng, ic, b, q):
                eng.dma_start(
                    out=xt_sb[ic, b, q][:],
                    in_=xt_d[ic, :, b * NB * S + q * 2 * S:
                             b * NB * S + (q + 1) * 2 * S])
            nc.sync.dma_start(out=m_sb[0, 0, 0][:], in_=m_d[0, :, 0:512])
            xt_dma(nc.sync, 0, 0, 0)
            nc.sync.dma_start(out=m_sb[1, 0, 0][:], in_=m_d[1, :, 0:512])
            xt_dma(nc.sync, 1, 0, 0)
            nc.sync.dma_start(out=m_sb[0, 0, 1][:], in_=m_d[0, :, 512:768])
            nc.sync.dma_start(out=m_sb[1, 0, 1][:], in_=m_d[1, :, 512:768])
            xt_dma(nc.sync, 0, 2, 0)
            xt_dma(nc.sync, 1, 2, 0)
            xt_dma(nc.gpsimd, 0, 1, 0)
            xt_dma(nc.gpsimd, 1, 1, 0)
            nc.gpsimd.dma_start(out=w_sb[0][:], in_=w_d[0])
            xt_dma(nc.gpsimd, 0, 3, 0)
            xt_dma(nc.gpsimd, 1, 3, 0)
            nc.gpsimd.dma_start(out=w_sb[1][:], in_=w_d[1])
            for q in range(1, NB // 2):
                for b in range(B):
                    for ic in range(2):
                        xt_dma(nc.gpsimd, ic, b, q)
                nc.gpsimd.dma_start(out=w_sb[2 * q][:], in_=w_d[2 * q])
                nc.gpsimd.dma_start(out=w_sb[2 * q + 1][:], in_=w_d[2 * q + 1])
            for ic in range(2):
                for ch, lo, hi in ((0, 768, 1280), (1, 1280, 1536)):
                    nc.gpsimd.dma_start(out=m_sb[ic, 1, ch][:],
                                        in_=m_d[ic, :, lo:hi])

            warm = persist.tile([S, 256], mybir.dt.bfloat16, tag="warm",
                                name="warm_sb")
            nc.vector.memset(warm[:], 0.0)
            for wi in range(8):
                pwarm = poutp.tile([S, 256], F32, tag="pout", name=f"pwarm{wi}")
                nc.tensor.matmul(pwarm[:], lhsT=warm[:, 0:128],
                                 rhs=warm[:], start=True, stop=True)

            for rep in range(reps):
                for oh in range(2):
                    y_tiles = []
                    for lb in range(NB):
                        # ---- projection: Y[lb] for all b, both signs ----
                        y_t = ypool.tile([S, 3072], F32R, tag="y",
                                         name=f"y_{rep}_{oh}_{lb}")
                        y_tiles.append(y_t)
                        for b in range(B):
                            py = pyp.tile([S, 768], F32, tag="py",
                                          name=f"py_{rep}_{oh}_{lb}_{b}")
                            for c0, c1 in ((0, 512), (512, 768)):
                                for ic in range(2):
                                    nc.tensor.matmul(
                                        py[:, c0:c1],
                                        lhsT=xt_sb[ic, b, lb // 2]
                                             [:, (lb % 2) * S:(lb % 2 + 1) * S],
                                        rhs=m_sb[ic, oh, 0 if c0 == 0 else 1][:, 0:c1 - c0],
                                        start=(ic == 0), stop=(ic == 1),
                                    )
                            # scatter (sign,kp,o) -> y col kp*1024+sign*512+b*128+o
                            src = py[:].rearrange("p (s k o) -> p k s o",
                                                  s=2, k=KPC)
                            dst = y_t[:].rearrange(
                                "p (k s bb o) -> p k s bb o",
                                k=KPC, s=2, bb=B, o=S)[:, :, :, b, :]
                            if b % 2 == 0:
                                nc.vector.tensor_copy(out=dst, in_=src)
                            else:
                                nc.scalar.copy(out=dst, in_=src)
                        # ---- conv accumulation into out block lb ----
                        pout = poutp.tile([S, 512], F32, tag="pout",
                                          name=f"pout_{rep}_{oh}_{lb}")
                        n_mm = 6 * (lb + 1)
                        i_mm = 0
                        for tb in range(lb + 1):
                            d = lb - tb
                            for kp in range(KPC):
                                for sg in range(2):
                                    nc.tensor.matmul(
                                        pout[:, 0:512],
                                        lhsT=w_sb[d][:, kp * 256 + sg * 128:
                                                     kp * 256 + sg * 128 + 128],
                                        rhs=y_tiles[tb][:, kp * 1024 + sg * 512:
                                                        kp * 1024 + sg * 512 + 512],
                                        start=(i_mm == 0),
                                        stop=(i_mm == n_mm - 1),
                                    )
                                    i_mm += 1
                        ost = ostage.tile([S, 512], F32, tag="ost",
                                          name=f"ost_{rep}_{oh}_{lb}")
                        nc.vector.tensor_copy(out=ost[:], in_=pout[:])
                        nc.sync.dma_start(out=out_d[oh, lb], in_=ost[:])
    nc.finalize()
    return nc


def _host_pack(x, phi, M_phi_plus, M_phi_minus):
    """Build host-side packed arrays; returns (xt, mcat_percore, w_percore)."""
    x = np.ascontiguousarray(x, dtype=np.float32)
    phi = np.ascontiguousarray(phi, dtype=np.float32)
    Mp = np.ascontiguousarray(M_phi_plus, dtype=np.float32)
    Mm = np.ascontiguousarray(M_phi_minus, dtype=np.float32)

    # xt[ic, i, b*1024 + tb*128 + t] = x[b, tb*128+t, ic*128+i]
    xt = np.ascontiguousarray(x.transpose(2, 0, 1).reshape(2, S, B * L))

    # Toeplitz blocks: base = d*128 + l - t
    tt = np.arange(S)
    ll = np.arange(S)
    arg = ll[None, :] - tt[:, None]                      # [t, l]
    base = arg[None, :, :] + (np.arange(NB) * S)[:, None, None]  # [d, t, l]
    valid = base >= 0
    idx = np.clip(base, 0, L - 1)
    Wp = np.where(valid[..., None], phi[idx], 0.0)       # [d, t, l, K]
    par = np.where(base % 2 == 0, 1.0, -1.0).astype(np.float32)
    Wm = Wp * par[..., None]
    # per-core w[d, t, kp*256 + sign*128 + l]
    w_cores = []
    for c in range(N_CORES):
        ks = slice(KPC * c, KPC * (c + 1))
        wc = np.stack([Wp[..., ks], Wm[..., ks]], axis=-1)  # [d,t,l,kp,2]
        wc = wc.transpose(0, 1, 3, 4, 2).reshape(NB, S, 768)
        w_cores.append(np.ascontiguousarray(wc.astype(np.float32)))

    # mcat[ic, i, oh*768 + sign*384 + kp*128 + o] = M_sign[kg, ic*128+i, oh*128+o]
    m_cores = []
    for c in range(N_CORES):
        ks = slice(KPC * c, KPC * (c + 1))
        mp = Mp[ks].reshape(KPC, 2, S, 2, S)   # [kp, ic, i, oh, o]
        mm = Mm[ks].reshape(KPC, 2, S, 2, S)
        mc = np.stack([mp, mm], axis=0)        # [sign, kp, ic, i, oh, o]
        mc = mc.transpose(2, 3, 4, 0, 1, 5).reshape(2, S, 1536)
        m_cores.append(np.ascontiguousarray(mc.astype(np.float32)))

    return xt, m_cores, w_cores


def kernel(x, phi, M_phi_plus, M_phi_minus):
    if "nc" not in _cache:
        _cache["nc"] = _build_program()
    nc = _cache["nc"]

    xt, m_cores, w_cores = _host_pack(x, phi, M_phi_plus, M_phi_minus)
    in_maps = [
        {"xt": xt, "mcat": m_cores[c], "w": w_cores[c]}
        for c in range(N_CORES)
    ]
    res = None
    last_err = None
    for attempt in range(3):
        try:
            res = run_bass_kernel_spmd(nc, in_maps,
                                       core_ids=list(range(N_CORES)))
            break
        except Exception as e:  # transient device wedge: retry
            last_err = e
    if res is None:
        raise last_err
    # out[oh, lb, l, b*128+o]; sum over cores, then reassemble [b, l, o]
    acc = np.zeros((2, NB, S, B * S), dtype=np.float64)
    for om in res.results:
        acc += om["out"]
    acc = acc.reshape(2, NB, S, B, S)           # [oh, lb, l, b, o]
    out = acc.transpose(3, 1, 2, 0, 4).reshape(B, L, O)
    return np.ascontiguousarray(out.astype(np.float32))

